# revision 1
# baseline (speedup 1.0000x reference)
"""Causal multi-head self-attention on 8 TRN2 NeuronCores.

Sharding: batch (4) x head-group (2) -> 8 cores. Each core computes, for its
batch b and its 8 heads, the attention output projected through its slice of
Wo; the host sums the two partial outputs per batch.

Per-core layout (P = 128 partitions):
  xT   [1024, 2048] bf16  - x[b].T (d_model on partitions)
  wqT/wkT/wvT [1024, 512] bf16 - weight slices, transposed to [in, out]
  woT  [512, 1024] f32    - Wo[:, g*512:(g+1)*512].T
  QT/KT [512, 2048] bf16  - head-dim on partitions (pair p -> tile p)
  V    16 tiles [128, 520] bf16 - seq on partitions, per-head 65-col groups
                                  (64 V cols + a ones col for row sums)
  scores computed transposed: S.T[k, q] = K @ Q.T, per head-pair via
  row-group packing (head A rows 0-63, head B rows 64-127).
  exp on ACT with fused 1/sqrt(dk) scale; causal via window-trimmed matmuls
  plus one [128,128] lower-triangle mask on diagonal blocks.
  AV: O.T[65, q] += V_aug.T @ E.T accumulated over k-tiles in PSUM
  (row 64 = softmax denominators). Normalize via vector reciprocal +
  PE broadcast (ones selector matmul). Final projection in fp32r.
"""

import numpy as np
import ml_dtypes

import concourse.bass as bass
import concourse.tile as tile
from concourse import bacc, mybir
from concourse import bass_utils

F32 = mybir.dt.float32
F32R = mybir.dt.float32r
BF16 = mybir.dt.bfloat16
NPBF16 = ml_dtypes.bfloat16

B, S, D, H, DK = 4, 2048, 1024, 16, 64
HC = 8          # heads per core
NPAIR = 4       # head pairs per core
OC = 512        # output dims per core (= HC * DK)
KT_N = 16       # seq k-tiles of 128
SCH = 4         # seq chunks of 512
SCALE = 1.0 / np.sqrt(np.float32(DK))

_CACHE = {}


def _emit(nc, tc, dram):
    P = 128
    xT_d, wqT_d, wkT_d, wvT_d, woT_d, tri_d, sel_d, out_d = (
        dram["xT"], dram["wqT"], dram["wkT"], dram["wvT"], dram["woT"],
        dram["trimask"], dram["sel2"], dram["out"],
    )

    import contextlib
    ctx = contextlib.ExitStack()
    with ctx:
        # ---------------- persistent SBUF ----------------
        per = ctx.enter_context(tc.tile_pool(name="per", bufs=1))
        trimask = per.tile([P, P], BF16, tag="trimask", name="trimask")
        nc.sync.dma_start(trimask[:], tri_d[:, :])
        sel2 = per.tile([2, P], F32R, tag="sel2", name="sel2")
        nc.sync.dma_start(sel2[:], sel_d[:, :])

        QT = [per.tile([P, S], BF16, tag=f"QT{p}", name=f"QT{p}") for p in range(NPAIR)]
        KT = [per.tile([P, S], BF16, tag=f"KT{p}", name=f"KT{p}") for p in range(NPAIR)]
        V = [per.tile([P, HC * 65], BF16, tag=f"V{t}", name=f"V{t}") for t in range(KT_N)]
        OT = [per.tile([P, S], F32R, tag=f"OT{p}", name=f"OT{p}") for p in range(NPAIR)]
        woT = [per.tile([P, D], F32R, tag=f"woT{p}", name=f"woT{p}") for p in range(NPAIR)]

        epool = ctx.enter_context(tc.tile_pool(name="epool", bufs=6))
        stg = ctx.enter_context(tc.tile_pool(name="stg", bufs=1))

        with tc.tile_pool(name="proj_in", bufs=1) as proj_in, \
             tc.tile_pool(name="wqk", bufs=2) as wqk_pool, \
             tc.tile_pool(name="ps", bufs=1, space="PSUM") as psp:
            xT = [proj_in.tile([P, S], BF16, tag=f"xT{k}", name=f"xT{k}") for k in range(8)]
            wvT = [proj_in.tile([P, OC], BF16, tag=f"wvT{k}", name=f"wvT{k}") for k in range(8)]
            for k in range(8):
                nc.sync.dma_start(wvT[k][:], wvT_d[k * P:(k + 1) * P, :])
            # xT in column-quarters: the first V-projection groups only need
            # the leading columns, so they start before the full 4MB lands
            for q in range(4):
                for k in range(8):
                    nc.sync.dma_start(
                        xT[k][:, q * 512:(q + 1) * 512],
                        xT_d[k * P:(k + 1) * P, q * 512:(q + 1) * 512])

            # PSUM budget (8 banks of [128,512]f32):
            #   pp: 2 x [128,512]  = 2 banks (projection accumulators)
            #   s:  2 x [128,1024] = 4 banks (scores, both heads)
            #   o:  1 x [128,1024] = 2 banks (output accum, both heads)
            def pp_tile():
                return psp.tile([P, OC], F32, tag="pp", bufs=2, name="pp")

            # ----- V projection: V[st] = x[st*128:+128, :] @ WvT, seq on partitions
            for st in range(KT_N):
                ps = pp_tile()
                for k in range(8):
                    nc.tensor.matmul(
                        ps[:], xT[k][:, st * P:(st + 1) * P], wvT[k][:],
                        start=(k == 0), stop=(k == 7),
                    )
                v3 = V[st].rearrange("p (h d) -> p h d", d=65)
                nc.vector.tensor_copy(
                    v3[:, :, 0:64], ps[:].rearrange("p (h d) -> p h d", d=64)
                )
                nc.gpsimd.memset(v3[:, :, 64:65], 1.0)

            # ----- QK projections + attention, pair by pair.
            # The PE executes its stream in order and the attention j-loop is
            # ACT(exp)-bound, so projection / final-projection matmuls are
            # interleaved as per-MM filler units inside the j-loop.
            def proj_units(p):
                units = []
                load_units = []
                for (wd, dst) in ((wqT_d, QT[p]), (wkT_d, KT[p])):
                    def load_w(wd=wd):
                        wsl = wqk_pool.tile([P, 8 * P], BF16, tag="wsl",
                                            name="wsl")
                        nc.sync.dma_start(
                            wsl[:].rearrange("p (k c) -> p k c", c=P),
                            wd[:, p * P:(p + 1) * P].rearrange(
                                "(k p) c -> p k c", p=P),
                        )
                        return wsl
                    wsl_box = []
                    load_units.append(lambda wsl_box=wsl_box, load_w=load_w:
                                      wsl_box.append(load_w()))
                    for sc in range(SCH):
                        ps_box = []
                        for k in range(8):
                            def mm(k=k, sc=sc, ps_box=ps_box,
                                   wsl_box=wsl_box):
                                if k == 0:
                                    ps_box.append(pp_tile())
                                nc.tensor.matmul(
                                    ps_box[0][:],
                                    wsl_box[0][:, k * P:(k + 1) * P],
                                    xT[k][:, sc * 512:(sc + 1) * 512],
                                    start=(k == 0), stop=(k == 7),
                                )
                            units.append(mm)
                        def cp(sc=sc, ps_box=ps_box, dst=dst):
                            nc.vector.tensor_copy(
                                dst[:, sc * 512:(sc + 1) * 512], ps_box[0][:])
                        units.append(cp)
                # both weight-slice DMAs lead the unit stream so neither
                # projection's first matmul waits on its load
                return iter(load_units + units)

            def final_units(cc):
                units = []
                for t in range(4 * cc, 4 * cc + 4):
                    for oc in range(2):
                        ps_box = []
                        for p4 in range(NPAIR):
                            def mm(p4=p4, t=t, oc=oc, ps_box=ps_box):
                                if p4 == 0:
                                    ps_box.append(
                                        psp.tile([P, OC], F32, tag="pp",
                                                 bufs=2, name="pp"))
                                nc.tensor.matmul(
                                    ps_box[0][:],
                                    OT[p4][:, t * P:(t + 1) * P],
                                    woT[p4][:, oc * 512:(oc + 1) * 512],
                                    start=(p4 == 0), stop=(p4 == NPAIR - 1),
                                )
                            units.append(mm)
                        def cp(t=t, oc=oc, ps_box=ps_box):
                            ostg = stg.tile([P, 512], F32, tag="ostg", bufs=4,
                                            name="ostg")
                            nc.scalar.copy(ostg[:], ps_box[0][:])
                            nc.sync.dma_start(
                                out_d[t * P:(t + 1) * P,
                                      oc * 512:(oc + 1) * 512],
                                ostg[:],
                            )
                        units.append(cp)
                return iter(units)

            norm_q = []
            # pair 0 projections run up front, undiluted
            for fn in proj_units(0):
                fn()
            for p in range(NPAIR):
                if p < NPAIR - 1:
                    pu = proj_units(p + 1)
                    fillers = [pu, pu, pu, pu]
                else:
                    for pq in range(NPAIR):
                        nc.sync.dma_start(
                            woT[pq][:], woT_d[pq * P:(pq + 1) * P, :])
                    fillers = [iter(()), final_units(0), final_units(1),
                               final_units(2)]
                _attention_pair(nc, tc, psp, epool, stg, p, QT, KT, V, OT,
                                sel2, trimask, norm_q, fillers)
            for fn in norm_q:
                fn()
            for fn in final_units(3):
                fn()

def _attention_pair(nc, tc, psp, epool, stg, p, QT, KT, V, OT, sel2,
                    trimask, norm_q, fillers):
    P = 128

    def fill(it, n):
        for _ in range(n):
            fn = next(it, None)
            if fn is None:
                return
            fn()

    for c in range(SCH):
        filler = fillers[c]
        if norm_q:
            norm_q.pop(0)()
        o2 = psp.tile([P, 1024], F32, tag="o", bufs=1, name="o2")
        njt = 4 * c + 4
        pend_av = []
        for j in range(njt):
            d = j - 4 * c
            w = d * P if d >= 0 else 0
            s2 = psp.tile([P, 1024], F32, tag="s", bufs=2, name="s2")
            for hh in range(2):
                nc.tensor.matmul(
                    s2[:, hh * 512 + w: hh * 512 + 512],
                    KT[p][hh * 64:(hh + 1) * 64, j * P:(j + 1) * P],
                    QT[p][hh * 64:(hh + 1) * 64, c * 512 + w:(c + 1) * 512],
                    start=True, stop=True,
                )
            e2 = epool.tile([P, 1024], BF16, tag="e", name="e2")
            nwid = 512 - w
            nc.scalar.activation(
                e2[:].rearrange("p (h q) -> p h q", h=2)[:, :, w:512],
                s2[:].rearrange("p (h q) -> p h q", h=2)[:, :, w:512],
                mybir.ActivationFunctionType.Exp,
                scale=float(SCALE),
            )
            if d >= 0:
                for hh in range(2):
                    blk = e2[:, hh * 512 + w: hh * 512 + w + P]
                    nc.vector.tensor_mul(blk, blk, trimask[:])
            fill(filler, 2)
            if len(pend_av) == 2:
                pend_av.pop(0)()

            def av(j=j, w=w, e2=e2, o2=o2, njt=njt):
                for hh in range(2):
                    head = 2 * p + hh
                    nc.tensor.matmul(
                        o2[0:65, hh * 512 + w: hh * 512 + 512],
                        V[j][:, head * 65: head * 65 + 65],
                        e2[:, hh * 512 + w: hh * 512 + 512],
                        start=(j == 0), stop=(j == njt - 1),
                    )
            pend_av.append(av)
        for fn in pend_av:
            fn()
        # PSUM -> SBUF staging (engines cannot shift partitions; DMA cannot
        # read PSUM), then SBUF->SBUF DMAs to place head B / sums rows.
        stage = stg.tile([65, 1024], F32R, tag="stage", bufs=2, name="stage")
        nc.scalar.copy(stage[:], o2[0:65, :])
        nc.sync.dma_start(
            OT[p][0:64, c * 512:(c + 1) * 512], stage[0:64, 0:512])
        nc.sync.dma_start(
            OT[p][64:128, c * 512:(c + 1) * 512], stage[0:64, 512:1024])
        # Hop the two sums rows (partition 64 of stage) down to
        # partitions 0-1 so the broadcast matmul sees aligned operands.
        sb2 = stg.tile([2, 512], F32R, tag="sb2", bufs=3, name="sb2")
        for hh in range(2):
            nc.sync.dma_start(
                sb2[hh:hh + 1, :], stage[64:65, hh * 512:(hh + 1) * 512])

        def _norm(sb2=sb2, p=p, c=c):
            # Deferred by one chunk so the PE-stream position of the bc
            # matmul is far past its dependencies -- the in-order PE
            # never stalls on it. Broadcast raw sums, then invert +
            # multiply on DVE (no PE dependency on the reciprocal).
            bc = psp.tile([P, OC], F32, tag="pp", bufs=2, name="pp")
            nc.tensor.matmul(bc[:], sel2[:], sb2[:], start=True, stop=True)
            nc.vector.reciprocal(bc[:], bc[:])
            nc.vector.tensor_mul(
                OT[p][:, c * 512:(c + 1) * 512],
                OT[p][:, c * 512:(c + 1) * 512],
                bc[:],
            )
        norm_q.append(_norm)
        # drain leftover filler, but only when the next chunk doesn't
        # continue the same iterator (projection fillers span the pair)
        if c == SCH - 1 or fillers[c + 1] is not filler:
            fill(filler, 10 ** 6)


def _build():
    if "nc" in _CACHE:
        return _CACHE["nc"]
    nc = bacc.Bacc("TRN2", target_bir_lowering=False, debug=False)
    dram = {
        "xT": nc.dram_tensor("xT", [D, S], BF16, kind="ExternalInput").ap(),
        "wqT": nc.dram_tensor("wqT", [D, OC], BF16, kind="ExternalInput").ap(),
        "wkT": nc.dram_tensor("wkT", [D, OC], BF16, kind="ExternalInput").ap(),
        "wvT": nc.dram_tensor("wvT", [D, OC], BF16, kind="ExternalInput").ap(),
        "woT": nc.dram_tensor("woT", [OC, D], F32R, kind="ExternalInput").ap(),
        "trimask": nc.dram_tensor("trimask", [128, 128], BF16,
                                  kind="ExternalInput").ap(),
        "sel2": nc.dram_tensor("sel2", [2, 128], F32R,
                               kind="ExternalInput").ap(),
        "out": nc.dram_tensor("out", [S, D], F32, kind="ExternalOutput").ap(),
    }
    with tile.TileContext(nc) as tc:
        _emit(nc, tc, dram)
    nc.compile()
    _CACHE["nc"] = nc
    return nc


def make_in_maps(x, Wq, Wk, Wv, Wo):
    x = np.asarray(x, np.float32)
    Wq = np.asarray(Wq, np.float32)
    Wk = np.asarray(Wk, np.float32)
    Wv = np.asarray(Wv, np.float32)
    Wo = np.asarray(Wo, np.float32)
    tri = np.tril(np.ones((128, 128), np.float32)).T.astype(NPBF16)
    sel = np.zeros((2, 128), np.float32)
    sel[0, 0:64] = 1.0
    sel[1, 64:128] = 1.0
    in_maps = []
    for c in range(8):
        b, g = divmod(c, 2)
        sl = slice(g * OC, (g + 1) * OC)
        in_maps.append({
            "xT": np.ascontiguousarray(x[b].T).astype(NPBF16),
            "wqT": np.ascontiguousarray(Wq[sl, :].T).astype(NPBF16),
            "wkT": np.ascontiguousarray(Wk[sl, :].T).astype(NPBF16),
            "wvT": np.ascontiguousarray(Wv[sl, :].T).astype(NPBF16),
            "woT": np.ascontiguousarray(Wo[:, sl].T).astype(np.float32),
            "trimask": tri,
            "sel2": sel,
        })
    return in_maps


def combine(results):
    parts = [results[c]["out"] for c in range(8)]
    return np.stack([parts[2 * b] + parts[2 * b + 1] for b in range(B)])


def kernel(**inputs):
    nc = _build()
    in_maps = make_in_maps(inputs["x"], inputs["Wq"], inputs["Wk"],
                           inputs["Wv"], inputs["Wo"])
    res = bass_utils.run_bass_kernel_spmd(nc, in_maps, core_ids=list(range(8)))
    return combine(res.results)


def run_traced(**inputs):
    nc = _build()
    in_maps = make_in_maps(inputs["x"], inputs["Wq"], inputs["Wk"],
                           inputs["Wv"], inputs["Wo"])
    res = bass_utils.run_bass_kernel_spmd(
        nc, in_maps, core_ids=list(range(8)), trace=True)
    return combine(res.results), res



# revision 10
# speedup vs baseline: 1.0336x; 1.0336x over previous
"""Causal multi-head self-attention on 8 TRN2 NeuronCores.

Sharding: batch (4) x head-group (2) -> 8 cores. Each core computes, for its
batch b and its 8 heads, the attention output projected through its slice of
Wo; the host sums the two partial outputs per batch.

Per-core layout (P = 128 partitions):
  xT   [1024, 2048] bf16  - x[b].T (d_model on partitions)
  wqT/wkT/wvT [1024, 512] bf16 - weight slices, transposed to [in, out]
  woT  [512, 1024] f32    - Wo[:, g*512:(g+1)*512].T
  QT/KT [512, 2048] bf16  - head-dim on partitions (pair p -> tile p)
  V    16 tiles [128, 520] bf16 - seq on partitions, per-head 65-col groups
                                  (64 V cols + a ones col for row sums)
  scores computed transposed: S.T[k, q] = K @ Q.T, per head-pair via
  row-group packing (head A rows 0-63, head B rows 64-127).
  exp on ACT with fused 1/sqrt(dk) scale; causal via window-trimmed matmuls
  plus one [128,128] lower-triangle mask on diagonal blocks.
  AV: O.T[65, q] += V_aug.T @ E.T accumulated over k-tiles in PSUM
  (row 64 = softmax denominators). Normalize via fast vector reciprocal
  on the [2,512] sums + PE broadcast of the inverted sums (bf16).
  Final projection all-bf16 so every LDWEIGHTS gets FWL.
"""

import numpy as np
import ml_dtypes

import concourse.bass as bass
import concourse.tile as tile
from concourse import bacc, mybir
from concourse import bass_utils

F32 = mybir.dt.float32
F32R = mybir.dt.float32r
BF16 = mybir.dt.bfloat16
NPBF16 = ml_dtypes.bfloat16

B, S, D, H, DK = 4, 2048, 1024, 16, 64
HC = 8          # heads per core
NPAIR = 4       # head pairs per core
OC = 512        # output dims per core (= HC * DK)
KT_N = 16       # seq k-tiles of 128
SCH = 4         # seq chunks of 512
SCALE = 1.0 / np.sqrt(np.float32(DK))

_CACHE = {}


def _emit(nc, tc, dram):
    P = 128
    xT_d, wqT_d, wkT_d, wvT_d, woT_d, tri_d, sel_d, out_d = (
        dram["xT"], dram["wqT"], dram["wkT"], dram["wvT"], dram["woT"],
        dram["trimask"], dram["sel2"], dram["out"],
    )

    import contextlib
    ctx = contextlib.ExitStack()
    with ctx:
        # ---------------- persistent SBUF ----------------
        per = ctx.enter_context(tc.tile_pool(name="per", bufs=1))
        trimask = per.tile([P, P], BF16, tag="trimask", name="trimask")
        nc.sync.dma_start(trimask[:], tri_d[:, :])
        sel2 = per.tile([2, P], BF16, tag="sel2", name="sel2")
        nc.sync.dma_start(sel2[:], sel_d[:, :])

        QT = [per.tile([P, S], BF16, tag=f"QT{p}", name=f"QT{p}") for p in range(NPAIR)]
        KT = [per.tile([P, S], BF16, tag=f"KT{p}", name=f"KT{p}") for p in range(NPAIR)]
        V = [per.tile([P, HC * 65], BF16, tag=f"V{t}", name=f"V{t}") for t in range(KT_N)]
        OT = [per.tile([P, S], BF16, tag=f"OT{p}", name=f"OT{p}") for p in range(NPAIR)]
        woT = [per.tile([P, D], BF16, tag=f"woT{p}", name=f"woT{p}") for p in range(NPAIR)]

        epool = ctx.enter_context(tc.tile_pool(name="epool", bufs=6))
        stg = ctx.enter_context(tc.tile_pool(name="stg", bufs=1))

        with tc.tile_pool(name="proj_in", bufs=1) as proj_in, \
             tc.tile_pool(name="wqk", bufs=2) as wqk_pool, \
             tc.tile_pool(name="ps", bufs=1, space="PSUM") as psp:
            xT = [proj_in.tile([P, S], BF16, tag=f"xT{k}", name=f"xT{k}") for k in range(8)]
            wvT = [proj_in.tile([P, OC], BF16, tag=f"wvT{k}", name=f"wvT{k}") for k in range(8)]
            # interleave wvT with the first xT strips so the k=0 V matmul can
            # issue after ~0.6MB instead of waiting for the full 2MB
            for k in range(8):
                nc.sync.dma_start(wvT[k][:], wvT_d[k * P:(k + 1) * P, :])
                nc.sync.dma_start(
                    xT[k][:, 0:256], xT_d[k * P:(k + 1) * P, 0:256])
            for k in range(8):
                nc.sync.dma_start(
                    xT[k][:, 256:512], xT_d[k * P:(k + 1) * P, 256:512])
            for q in range(1, 4):
                for k in range(8):
                    nc.sync.dma_start(
                        xT[k][:, q * 512:(q + 1) * 512],
                        xT_d[k * P:(k + 1) * P, q * 512:(q + 1) * 512])

            # PSUM budget (8 banks of [128,512]f32):
            #   pp: 2 x [128,512]  = 2 banks (projection accumulators)
            #   s:  2 x [128,1024] = 4 banks (scores, both heads)
            #   o:  1 x [128,1024] = 2 banks (output accum, both heads)
            def pp_tile():
                return psp.tile([P, OC], F32, tag="pp", bufs=2, name="pp")

            # ----- V projection: V[st] = x[st*128:+128, :] @ WvT, seq on partitions
            for st in range(KT_N):
                ps = pp_tile()
                for k in range(8):
                    nc.tensor.matmul(
                        ps[:], xT[k][:, st * P:(st + 1) * P], wvT[k][:],
                        start=(k == 0), stop=(k == 7),
                    )
                v3 = V[st].rearrange("p (h d) -> p h d", d=65)
                nc.vector.tensor_copy(
                    v3[:, :, 0:64], ps[:].rearrange("p (h d) -> p h d", d=64)
                )
                nc.gpsimd.memset(v3[:, :, 64:65], 1.0)

            # ----- QK projections + attention, pair by pair.
            # The PE executes its stream in order and the attention j-loop is
            # ACT(exp)-bound, so projection / final-projection matmuls are
            # interleaved as per-MM filler units inside the j-loop.
            def proj_units(p):
                units = []
                load_units = []
                for (wd, dst) in ((wqT_d, QT[p]), (wkT_d, KT[p])):
                    def load_w(wd=wd):
                        wsl = wqk_pool.tile([P, 8 * P], BF16, tag="wsl",
                                            name="wsl")
                        nc.sync.dma_start(
                            wsl[:].rearrange("p (k c) -> p k c", c=P),
                            wd[:, p * P:(p + 1) * P].rearrange(
                                "(k p) c -> p k c", p=P),
                        )
                        return wsl
                    wsl_box = []
                    load_units.append(lambda wsl_box=wsl_box, load_w=load_w:
                                      wsl_box.append(load_w()))
                    for sc in range(SCH):
                        ps_box = []
                        for k in range(8):
                            def mm(k=k, sc=sc, ps_box=ps_box,
                                   wsl_box=wsl_box):
                                if k == 0:
                                    ps_box.append(pp_tile())
                                nc.tensor.matmul(
                                    ps_box[0][:],
                                    wsl_box[0][:, k * P:(k + 1) * P],
                                    xT[k][:, sc * 512:(sc + 1) * 512],
                                    start=(k == 0), stop=(k == 7),
                                )
                            units.append(mm)
                        def cp(sc=sc, ps_box=ps_box, dst=dst):
                            nc.vector.tensor_copy(
                                dst[:, sc * 512:(sc + 1) * 512], ps_box[0][:])
                        units.append(cp)
                # both weight-slice DMAs lead the unit stream so neither
                # projection's first matmul waits on its load
                return iter(load_units + units)

            def final_units(cc):
                units = []
                for t in range(4 * cc, 4 * cc + 4):
                    for oc in range(2):
                        ps_box = []
                        for p4 in range(NPAIR):
                            def mm(p4=p4, t=t, oc=oc, ps_box=ps_box):
                                if p4 == 0:
                                    ps_box.append(
                                        psp.tile([P, OC], F32, tag="pp",
                                                 bufs=2, name="pp"))
                                nc.tensor.matmul(
                                    ps_box[0][:],
                                    OT[p4][:, t * P:(t + 1) * P],
                                    woT[p4][:, oc * 512:(oc + 1) * 512],
                                    start=(p4 == 0), stop=(p4 == NPAIR - 1),
                                )
                            units.append(mm)
                        def cp(t=t, oc=oc, ps_box=ps_box):
                            ostg = stg.tile([P, 512], F32, tag="ostg", bufs=4,
                                            name="ostg")
                            nc.vector.tensor_copy(ostg[:], ps_box[0][:])
                            nc.sync.dma_start(
                                out_d[t * P:(t + 1) * P,
                                      oc * 512:(oc + 1) * 512],
                                ostg[:],
                            )
                        units.append(cp)
                return iter(units)

            norm_q = []
            # pair 0 projections run up front, undiluted
            for fn in proj_units(0):
                fn()
            for p in range(NPAIR):
                if p < NPAIR - 1:
                    pu = proj_units(p + 1)
                    fillers = [pu, pu, pu, pu]
                else:
                    for pq in range(NPAIR):
                        nc.sync.dma_start(
                            woT[pq][:], woT_d[pq * P:(pq + 1) * P, :])
                    fillers = [iter(()), final_units(0), final_units(1),
                               final_units(2)]
                _attention_pair(nc, tc, psp, epool, stg, p, QT, KT, V, OT,
                                sel2, trimask, norm_q, fillers)
            for fn in norm_q:
                fn()
            for fn in final_units(3):
                fn()

def _attention_pair(nc, tc, psp, epool, stg, p, QT, KT, V, OT, sel2,
                    trimask, norm_q, fillers):
    P = 128

    def fill(it, n):
        for _ in range(n):
            fn = next(it, None)
            if fn is None:
                return
            fn()

    for c in range(SCH):
        filler = fillers[c]
        if norm_q:
            norm_q.pop(0)()
        o2 = psp.tile([P, 1024], F32, tag="o", bufs=1, name="o2")
        njt = 4 * c + 4
        pend_av = []
        for j in range(njt):
            d = j - 4 * c
            w = d * P if d >= 0 else 0
            s2 = psp.tile([P, 1024], F32, tag="s", bufs=2, name="s2")
            for hh in range(2):
                nc.tensor.matmul(
                    s2[:, hh * 512 + w: hh * 512 + 512],
                    KT[p][hh * 64:(hh + 1) * 64, j * P:(j + 1) * P],
                    QT[p][hh * 64:(hh + 1) * 64, c * 512 + w:(c + 1) * 512],
                    start=True, stop=True,
                )
            e2 = epool.tile([P, 1024], BF16, tag="e", name="e2")
            nwid = 512 - w
            nc.scalar.activation(
                e2[:].rearrange("p (h q) -> p h q", h=2)[:, :, w:512],
                s2[:].rearrange("p (h q) -> p h q", h=2)[:, :, w:512],
                mybir.ActivationFunctionType.Exp,
                scale=float(SCALE),
            )
            if d >= 0:
                for hh in range(2):
                    blk = e2[:, hh * 512 + w: hh * 512 + w + P]
                    nc.gpsimd.tensor_mul(blk, blk, trimask[:])
            fill(filler, 2)
            if len(pend_av) == 2:
                pend_av.pop(0)()

            def av(j=j, w=w, e2=e2, o2=o2, njt=njt):
                for hh in range(2):
                    head = 2 * p + hh
                    nc.tensor.matmul(
                        o2[0:65, hh * 512 + w: hh * 512 + 512],
                        V[j][:, head * 65: head * 65 + 65],
                        e2[:, hh * 512 + w: hh * 512 + 512],
                        start=(j == 0), stop=(j == njt - 1),
                    )
            pend_av.append(av)
        for fn in pend_av:
            fn()
        # PSUM -> SBUF staging (engines cannot shift partitions; DMA cannot
        # read PSUM), then SBUF->SBUF DMAs to place head B / sums rows.
        # Head dims evac to bf16 (feeds bf16 OT/final matmul); sums row
        # stays fp32 for the reciprocal.
        stage = stg.tile([64, 1024], BF16, tag="stage", bufs=2, name="stage")
        nc.vector.tensor_copy(stage[:], o2[0:64, :])
        sums = stg.tile([1, 1024], F32, tag="sums", bufs=2, name="sums")
        nc.vector.tensor_copy(sums[:], o2[64:65, :])
        nc.sync.dma_start(
            OT[p][0:64, c * 512:(c + 1) * 512], stage[0:64, 0:512])
        nc.sync.dma_start(
            OT[p][64:128, c * 512:(c + 1) * 512], stage[0:64, 512:1024])
        # Hop the two sums rows (partition 64 of stage) down to
        # partitions 0-1 so the broadcast matmul sees aligned operands.
        sb2 = stg.tile([2, 512], F32, tag="sb2", bufs=3, name="sb2")
        for hh in range(2):
            nc.sync.dma_start(
                sb2[hh:hh + 1, :], sums[0:1, hh * 512:(hh + 1) * 512])
        rb2 = stg.tile([2, 512], BF16, tag="rb2", bufs=3, name="rb2")
        nc.vector.reciprocal_approx_fast(sb2[:], sb2[:])
        nc.vector.tensor_copy(rb2[:], sb2[:])

        def _norm(rb2=rb2, p=p, c=c):
            # Deferred by one chunk so the PE-stream position of the bc
            # matmul is far past its dependencies -- the in-order PE
            # never stalls on it. Broadcast the (pre-inverted) sums, then
            # multiply on DVE.
            bc = psp.tile([P, OC], F32, tag="pp", bufs=2, name="pp")
            nc.tensor.matmul(bc[:], sel2[:], rb2[:], start=True, stop=True)
            nc.vector.tensor_mul(
                OT[p][:, c * 512:(c + 1) * 512],
                OT[p][:, c * 512:(c + 1) * 512],
                bc[:],
            )
        norm_q.append(_norm)
        # drain leftover filler, but only when the next chunk doesn't
        # continue the same iterator (projection fillers span the pair)
        if c == SCH - 1 or fillers[c + 1] is not filler:
            fill(filler, 10 ** 6)


def _build():
    if "nc" in _CACHE:
        return _CACHE["nc"]
    nc = bacc.Bacc("TRN2", target_bir_lowering=False, debug=False)
    dram = {
        "xT": nc.dram_tensor("xT", [D, S], BF16, kind="ExternalInput").ap(),
        "wqT": nc.dram_tensor("wqT", [D, OC], BF16, kind="ExternalInput").ap(),
        "wkT": nc.dram_tensor("wkT", [D, OC], BF16, kind="ExternalInput").ap(),
        "wvT": nc.dram_tensor("wvT", [D, OC], BF16, kind="ExternalInput").ap(),
        "woT": nc.dram_tensor("woT", [OC, D], BF16, kind="ExternalInput").ap(),
        "trimask": nc.dram_tensor("trimask", [128, 128], BF16,
                                  kind="ExternalInput").ap(),
        "sel2": nc.dram_tensor("sel2", [2, 128], BF16,
                               kind="ExternalInput").ap(),
        "out": nc.dram_tensor("out", [S, D], F32, kind="ExternalOutput").ap(),
    }
    with tile.TileContext(nc) as tc:
        _emit(nc, tc, dram)
    nc.compile()
    _CACHE["nc"] = nc
    return nc


def make_in_maps(x, Wq, Wk, Wv, Wo):
    x = np.asarray(x, np.float32)
    Wq = np.asarray(Wq, np.float32)
    Wk = np.asarray(Wk, np.float32)
    Wv = np.asarray(Wv, np.float32)
    Wo = np.asarray(Wo, np.float32)
    tri = np.tril(np.ones((128, 128), np.float32)).T.astype(NPBF16)
    sel = np.zeros((2, 128), NPBF16)
    sel[0, 0:64] = 1.0
    sel[1, 64:128] = 1.0
    in_maps = []
    for c in range(8):
        b, g = divmod(c, 2)
        sl = slice(g * OC, (g + 1) * OC)
        in_maps.append({
            "xT": np.ascontiguousarray(x[b].T).astype(NPBF16),
            "wqT": np.ascontiguousarray(Wq[sl, :].T).astype(NPBF16),
            "wkT": np.ascontiguousarray(Wk[sl, :].T).astype(NPBF16),
            "wvT": np.ascontiguousarray(Wv[sl, :].T).astype(NPBF16),
            "woT": np.ascontiguousarray(Wo[:, sl].T).astype(NPBF16),
            "trimask": tri,
            "sel2": sel,
        })
    return in_maps


def combine(results):
    parts = [results[c]["out"] for c in range(8)]
    return np.stack([parts[2 * b] + parts[2 * b + 1] for b in range(B)])


def kernel(**inputs):
    nc = _build()
    in_maps = make_in_maps(inputs["x"], inputs["Wq"], inputs["Wk"],
                           inputs["Wv"], inputs["Wo"])
    res = bass_utils.run_bass_kernel_spmd(nc, in_maps, core_ids=list(range(8)))
    return combine(res.results)


def run_traced(**inputs):
    nc = _build()
    in_maps = make_in_maps(inputs["x"], inputs["Wq"], inputs["Wk"],
                           inputs["Wv"], inputs["Wo"])
    res = bass_utils.run_bass_kernel_spmd(
        nc, in_maps, core_ids=list(range(8)), trace=True)
    return combine(res.results), res

